# revision 9
# baseline (speedup 1.0000x reference)
"""Bass/Trainium2 kernel for nn_BillehColumn (recurrent synaptic currents).

i_rec[b, post] = sum_e w[e] * z[b, pre[e]] * [post[e] == post],  output flat [B*N].

Strategy (8 NeuronCores, SPMD):
  - Shard the 10M synapses across 8 cores (edge sharding per the hint).
  - Host-side layout prep only: within each core's shard, group synapses by
    pre-neuron block (pre // 128) and pad each group to a multiple of 128 so a
    chunk of 128 synapses shares one z-block; replicate rec_z_buf per chunk
    (the hint's "replicated rec_z_buf"), and precompute index decompositions
    (pre % 128, post % 128, post // 128) as device-friendly dtypes.
  - Device: for each 128-synapse chunk, build the pre one-hot on DVE, PE-
    transpose it, matmul against the chunk's z block to gather z for both
    batches, scale the post one-hot by w*z, and accumulate i_rec[r, q] into
    PSUM via two binning matmuls (one per batch).  Partial [128, 391, 2]
    accumulators from the 8 cores are summed on the host (unshard).
"""

import numpy as np

import concourse.bass as bass
import concourse.bacc as bacc
import concourse.mybir as mybir
import concourse.tile as tile
from concourse.bass_utils import run_bass_kernel_spmd
import ml_dtypes

B = 2
N_NEURONS = 50000
N_SYNAPSES = 10_000_000
N_CORES = 8
P = 128
NQ = 391            # ceil(50000 / 128) post blocks
NQPAD = 392         # padded (post one-hot table width, even)
E_CORE = N_SYNAPSES // N_CORES


def _host_prepare(rec_z_buf, synapse_indices, weight_values):
    """Shard + layout prep. Returns (in_maps, nch) for the 8 cores."""
    z = np.asarray(rec_z_buf, dtype=np.float32)          # [2, 50000]
    syn = np.asarray(synapse_indices)                    # [10M, 2] int64
    w = np.asarray(weight_values, dtype=np.float32)      # [10M]

    post = syn[:, 0].astype(np.int32)
    pre = syn[:, 1].astype(np.int32)

    shards = []
    max_nch = 0
    for c in range(N_CORES):
        lo, hi = c * E_CORE, (c + 1) * E_CORE
        pr, po, wv = pre[lo:hi], post[lo:hi], w[lo:hi]
        # group by pre block (stable; this is a range-grouping, not a value sort)
        qpre = pr >> 7
        order = np.argsort(qpre, kind="stable")
        pr, po, wv, qpre = pr[order], po[order], wv[order], qpre[order]
        # pad each group to a multiple of 128 with null synapses (w = 0)
        counts = np.bincount(qpre, minlength=NQ)
        padded = (counts + P - 1) // P * P
        tot = int(padded.sum())
        nch = tot // P
        gstart = np.concatenate([[0], np.cumsum(padded)])[:-1]
        src_start = np.concatenate([[0], np.cumsum(counts)])[:-1]
        # destination slot of each (sorted) synapse
        dst = (np.arange(len(pr)) - src_start[qpre]) + gstart[qpre]
        pr_s = np.zeros(tot, np.int32)
        po_s = np.zeros(tot, np.int32)
        wv_s = np.zeros(tot, np.float32)
        pr_s[dst], po_s[dst], wv_s[dst] = pr, po, wv
        # chunk id -> pre block (constant within a chunk by construction)
        chunk_q = np.zeros(nch, np.int32)
        for q in range(NQ):
            if padded[q]:
                chunk_q[gstart[q] // P:(gstart[q] + padded[q]) // P] = q
        shards.append((pr_s, po_s, wv_s, chunk_q, nch))
        max_nch = max(max_nch, nch)

    nch = (max_nch + 31) // 32 * 32  # unroll-friendly
    in_maps = []
    for pr_s, po_s, wv_s, chunk_q, n0 in shards:
        tot = nch * P
        def pad(a, fill=0):
            out = np.full(tot, fill, a.dtype)
            out[:len(a)] = a
            return out
        pr_s, po_s, wv_s = pad(pr_s), pad(po_s), pad(wv_s)
        cq = np.zeros(nch, np.int32)
        cq[:len(chunk_q)] = chunk_q
        # synapse-per-partition layout: slot i -> [i % 128, i // 128]
        def lay(a, dt):
            return np.ascontiguousarray(a.reshape(nch, P).T).astype(dt)
        pp = lay((pr_s & 127).astype(np.float32), ml_dtypes.bfloat16)   # pre % 128 (bf16-exact)
        rr = lay((po_s & 127).astype(np.float32), ml_dtypes.bfloat16)   # post % 128
        qq = lay((po_s >> 7).astype(np.float32), np.float32)            # post // 128
        ww = lay(wv_s, np.float32)
        # replicated z block per chunk: zsel[p, t, b] = z[b, chunk_q[t]*128 + p]
        zpad = np.zeros((B, NQ * P), np.float32)
        zpad[:, :N_NEURONS] = np.asarray(rec_z_buf, np.float32)
        zblk = zpad.reshape(B, NQ, P)                                   # [b, q, p]
        zsel = np.ascontiguousarray(
            zblk[:, cq, :].transpose(2, 1, 0)                            # [p, t, b]
        ).astype(ml_dtypes.bfloat16).reshape(P, nch * B)
        in_maps.append({"pp": pp, "rr": rr, "qq": qq, "ww": ww, "zsel": zsel})
    return in_maps, nch


def _build_kernel(nch, unroll):
    nc = bacc.Bacc(None, target_bir_lowering=False)
    f32, bf16 = mybir.dt.float32, mybir.dt.bfloat16

    pp_d = nc.dram_tensor("pp", [P, nch], bf16, kind="ExternalInput")
    rr_d = nc.dram_tensor("rr", [P, nch], bf16, kind="ExternalInput")
    qq_d = nc.dram_tensor("qq", [P, nch], f32, kind="ExternalInput")
    ww_d = nc.dram_tensor("ww", [P, nch], f32, kind="ExternalInput")
    zsel_d = nc.dram_tensor("zsel", [P, nch * B], bf16, kind="ExternalInput")
    out_d = nc.dram_tensor("part", [P, NQPAD * B], f32, kind="ExternalOutput")

    n_iter = nch // unroll

    with tile.TileContext(nc) as tc:
        with tc.tile_pool(name="pool", bufs=1) as pool, \
             tc.tile_pool(name="psum", bufs=2, space="PSUM") as psum, \
             tc.tile_pool(name="psumT", bufs=2, space="PSUM") as psumT, \
             tc.tile_pool(name="psumG", bufs=2, space="PSUM") as psumG:
            pp_t = pool.tile([P, nch], bf16)
            rr_t = pool.tile([P, nch], bf16)
            qq_t = pool.tile([P, nch], f32)
            ww_t = pool.tile([P, nch], f32)
            zsel_t = pool.tile([P, nch * B], bf16)
            nc.sync.dma_start(pp_t[:], pp_d[:])
            nc.sync.dma_start(rr_t[:], rr_d[:])
            nc.sync.dma_start(qq_t[:], qq_d[:])
            nc.sync.dma_start(ww_t[:], ww_d[:])
            nc.sync.dma_start(zsel_t[:], zsel_d[:])

            # static tables
            iota128_b = pool.tile([P, P], bf16)      # iota along free dim
            iota392_f = pool.tile([P, NQPAD], f32)
            ident_b = pool.tile([P, P], bf16)
            nc.gpsimd.iota(iota128_b[:], pattern=[[1, P]], base=0, channel_multiplier=0, allow_small_or_imprecise_dtypes=True)
            nc.gpsimd.iota(iota392_f[:], pattern=[[1, NQPAD]], base=0, channel_multiplier=0, allow_small_or_imprecise_dtypes=True)
            from concourse.masks import make_identity
            make_identity(nc, ident_b[:])

            acc = pool.tile([P, NQPAD * B], f32)     # [r, q*2 + b]
            nc.vector.memset(acc[:], 0.0)

            def body(it):
                bin0 = psum.tile([P, NQPAD], f32, tag="bin0")
                bin1 = psum.tile([P, NQPAD], f32, tag="bin1")
                binp = [bin0, bin1]
                for u in range(unroll):
                    t = it * unroll + u if n_iter > 1 else u
                    # chunk column slices
                    pp_c = pp_t[:, bass.ts(t, 1)]
                    rr_c = rr_t[:, bass.ts(t, 1)]
                    qq_c = qq_t[:, bass.ts(t, 1)]
                    ww_c = ww_t[:, bass.ts(t, 1)]
                    z_c = zsel_t[:, bass.ts(t, B)]
                    # 1) pre one-hot, [k, p] orientation (k = synapse on partitions)
                    ohpT = pool.tile([P, P], bf16, tag="ohpT")
                    nc.vector.tensor_tensor(
                        out=ohpT[:], in0=iota128_b[:],
                        in1=pp_c.to_broadcast([P, P]),
                        op=mybir.AluOpType.is_equal)
                    # 2) transpose -> [p, k] in PSUM, copy to SBUF bf16
                    ohp_ps = psumT.tile([P, P], bf16, tag="ohp_ps")
                    nc.tensor.transpose(out=ohp_ps[:], in_=ohpT[:], identity=ident_b[:])
                    ohp = pool.tile([P, P], bf16, tag="ohp")
                    nc.scalar.copy(ohp[:], ohp_ps[:])
                    # 3) gather z for both batches: G[k, b] = sum_p ohp[p,k] * z[p,b]
                    g_ps = psumG.tile([P, B], f32, tag="g_ps")
                    nc.tensor.matmul(g_ps[:], lhsT=ohp[:], rhs=z_c, start=True, stop=True)
                    # 4) contributions c_b = w * G_b  (bf16)
                    c_t = pool.tile([P, B], bf16, tag="c_t")
                    nc.vector.tensor_scalar(
                        out=c_t[:], in0=g_ps[:], scalar1=ww_c, scalar2=None,
                        op0=mybir.AluOpType.mult)
                    # 5) post-q one-hot rhs [k, q]
                    qoh = pool.tile([P, NQPAD], bf16, tag="qoh")
                    nc.vector.tensor_tensor(
                        out=qoh[:], in0=iota392_f[:],
                        in1=qq_c.to_broadcast([P, NQPAD]),
                        op=mybir.AluOpType.is_equal)
                    eq_r = pool.tile([P, P], bf16, tag="eq_r")
                    nc.vector.tensor_tensor(
                        out=eq_r[:], in0=iota128_b[:],
                        in1=rr_c.to_broadcast([P, P]),
                        op=mybir.AluOpType.is_equal)
                    # 6) per-batch scaled post-r one-hot lhsT [k, r], then bin
                    for b in range(B):
                        lhs = pool.tile([P, P], bf16, tag=f"lhs{b}")
                        nc.vector.tensor_tensor(
                            out=lhs[:], in0=eq_r[:],
                            in1=c_t[:, b:b + 1].to_broadcast([P, P]),
                            op=mybir.AluOpType.mult)
                        nc.tensor.matmul(binp[b][:], lhsT=lhs[:], rhs=qoh[:],
                                         start=(u == 0), stop=(u == unroll - 1))
                # flush PSUM into the SBUF accumulator
                for b in range(B):
                    nc.vector.tensor_add(
                        out=acc[:].rearrange("p (q b) -> p b q", b=B)[:, b, :],
                        in0=acc[:].rearrange("p (q b) -> p b q", b=B)[:, b, :],
                        in1=binp[b][:])

            if n_iter > 1:
                with tc.For_i(0, n_iter, 1) as it:
                    body(it)
            else:
                body(0)

            nc.sync.dma_start(out_d[:], acc[:])
    nc.compile()
    return nc


_CACHE = {}
_TRACE = False
LAST_EXEC_NS = None


def kernel(rec_z_buf, synapse_indices, weight_values, n_post_neurons):
    n_post = int(n_post_neurons)
    in_maps, nch = _host_prepare(rec_z_buf, synapse_indices, weight_values)
    unroll = 32
    key = (nch, unroll)
    if key not in _CACHE:
        _CACHE[key] = _build_kernel(nch, unroll)
    nc = _CACHE[key]
    global LAST_EXEC_NS
    res = run_bass_kernel_spmd(nc, in_maps, core_ids=list(range(N_CORES)), trace=_TRACE)
    LAST_EXEC_NS = res.exec_time_ns
    # unshard: sum partials, reorder [r, q, b] -> [b, q*128 + r]
    total = np.zeros((P, NQPAD * B), np.float64)
    for r in res.results:
        total += r["part"].astype(np.float64)
    total = total.reshape(P, NQPAD, B)           # [r, q, b]
    i_rec = total.transpose(2, 1, 0).reshape(B, NQPAD * P)[:, :n_post]
    return np.ascontiguousarray(i_rec.reshape(-1)).astype(np.float32)
